# revision 7
# baseline (speedup 1.0000x reference)
"""HolE scorer kernel for 8 Trainium2 NeuronCores (Bass/Tile), fp8 edition.

Computation (reference):
    a = x @ W_e.T; b = y @ W_e.T; rr = r @ W_r.T          # (B, d)
    corr = irfft(rfft(a) * conj(rfft(b))) / d             # circular correlation
    out = sigmoid(sum(rr * corr, axis=1))                 # (B, 1)

Key identity used here: score_i = sum_d a[i,d] * psi[i,d] where
    psi = irfft(rfft(rr) * rfft(b)) / d   (circular convolution dual)
so the score is LINEAR in the per-core partial a's: the x-side GEMM needs
no collective at all - each core emits a partial score vector (1, B) and
the host sums 8 of them (the "unshard" step) and applies the sigmoid.

Strategy:
  - Tensor-parallel over entities: core c holds entity rows
    [c*12500, (c+1)*12500) of x.T/y.T/W_e.T (padded to 12544 = 49*256).
  - Both big GEMMs run in fp8 e4m3 with DoubleRow (double-pumped) matmuls:
    K=256 per instruction at the same 512-column stream rate as bf16.
    Inputs scaled (x*16, W_e*4096) to sit in e4m3's normal range; the
    1/65536^2 descale plus the irfft w_f/d^2 factors are folded into the
    host-side R = rfft(rr) computation.
  - Host pre-packs x/y/W_e shards into the exact SBUF tile layout
    (partition-major, contiguous per partition per group) so every stream
    DMA runs at HBM line rate; R = rfft(r @ W_r.T) is tiny and
    input-independent of the device dataflow, so it is computed on host.
  - y-side: partial b staged bf16 (stage writes on the idle GPSIMD SWDGE
    queue), one ReduceScatter(add) -> core c owns batch cols
    [128c, 128c+128).  B = rfft(b); P/Q complex product; psi via
    irfft-basis matmuls; AllGather psi.  The whole chain is emitted after
    the first k-group of the final x pass, where the RS is guaranteed
    done, so the PE never stalls on it.
  - x-side: partial a stays on-chip; per-core partial scores via
    elementwise mult with psi + ones-vector matmul; (1, B) f32 out per
    core; host sums partials and applies the sigmoid.
"""

import numpy as np
import ml_dtypes

import concourse.bass as bass
import concourse.tile as tile
from concourse import bacc, mybir
from concourse.alu_op_type import AluOpType
from concourse.bass_utils import run_bass_kernel_spmd

# Problem shapes (hardcoded per contract)
B = 1024            # batch
D = 512             # num_dim
E = 100000          # num_entities
R = 1000            # num_relations
NCORES = 8

E_SH = E // NCORES          # 12500 entities per core
KC = 98                     # 128-row k-chunks after padding (98*128 = 12544)
NPAIR = KC // 2             # 49 DoubleRow (K=256) chunks
E_PAD = KC * 128            # 12544
KG = 7                      # stream k-groups
KJ = KC // KG               # 14 chunks per group (7 pairs)
NF = D // 2 + 1             # 257 rfft bins
FC = 3                      # frequency chunks of 128
F_PAD = FC * 128            # 384
BC = B // NCORES            # 128 batch cols owned per core (tail sharding)

SX = 16.0                   # x/y fp8 scale
SW = 4096.0                 # W_e fp8 scale

BF16 = mybir.dt.bfloat16
F32 = mybir.dt.float32
FP8 = mybir.dt.float8e4
DR = mybir.MatmulPerfMode.DoubleRow

_cached = {}


def _host_consts():
    dd = np.arange(D, dtype=np.float64)[:, None]
    ff = np.arange(NF, dtype=np.float64)[None, :]
    ang = 2.0 * np.pi * dd * ff / D
    fr = np.cos(ang)                      # (D, NF)
    fi = -np.sin(ang)
    w = np.full(NF, 2.0); w[0] = 1.0; w[-1] = 1.0
    fold = w / (D * D) / (SX * SW) ** 2

    bf = ml_dtypes.bfloat16
    # d-major rfft basis, f padded to 384: fabD[d, ri, f]
    fabD = np.zeros((D, 2, F_PAD), dtype=bf)
    fabD[:, 0, :NF] = fr.astype(bf)
    fabD[:, 1, :NF] = fi.astype(bf)
    # f-major irfft basis: fabF[f, ri, d]
    fabF = np.zeros((F_PAD, 2, D), dtype=bf)
    fabF[:NF, 0, :] = fr.T.astype(bf)
    fabF[:NF, 1, :] = fi.T.astype(bf)
    return fabD, fabF, fr, fi, fold


def _build_program():
    nc = bacc.Bacc("TRN2", target_bir_lowering=False, debug=False,
                   num_devices=NCORES)

    # stream tensors pre-packed on host into tile layout:
    #   xT[n, g, p, j*512+q] = x.T[core_rows: (g*KJ+j)*128+p, n*512+q] (fp8)
    xT_d = nc.dram_tensor("xT", (2, KG, 128, KJ * 512), FP8,
                          kind="ExternalInput")
    yT_d = nc.dram_tensor("yT", (2, KG, 128, KJ * 512), FP8,
                          kind="ExternalInput")
    weT_d = nc.dram_tensor("weT", (KG, 128, KJ * D), FP8,
                           kind="ExternalInput")
    # host-computed R = rfft(r @ W_r.T) with w_f/d^2/descale folded,
    # f-major, own batch cols: R_d[p, ri, fc, j] = R_ri[fc*128+p, own j]
    R_d = nc.dram_tensor("Rh", (128, 2, FC, 128), BF16, kind="ExternalInput")
    fabD_d = nc.dram_tensor("fabD", (D, 2, F_PAD), BF16, kind="ExternalInput")
    fabF_d = nc.dram_tensor("fabF", (F_PAD, 2, D), BF16, kind="ExternalInput")
    ones_d = nc.dram_tensor("ones", (128, 1), BF16, kind="ExternalInput")
    out_d = nc.dram_tensor("out", (1, B), F32, kind="ExternalOutput")

    stage_y = nc.dram_tensor("stage_y", (NCORES, D, BC), BF16)
    rs_y = nc.dram_tensor("rs_y", (D, BC), BF16)
    ag_in = nc.dram_tensor("ag_in", (D, BC), BF16)
    ag_out = nc.dram_tensor("ag_out", (NCORES, D, BC), BF16,
                            addr_space="Shared")
    groups = [list(range(NCORES))]

    with tile.TileContext(nc) as tc:
        with (
            tc.tile_pool(name="weights", bufs=1) as wpool,
            tc.tile_pool(name="stream", bufs=5) as spool,
            tc.tile_pool(name="copies", bufs=4) as cpool,
            tc.tile_pool(name="tail", bufs=1) as tpool,
            tc.tile_pool(name="psum", bufs=4, space="PSUM") as ppool,
            tc.tile_pool(name="psum_small", bufs=4, space="PSUM") as qpool,
        ):
            # ---- resident W_e.T (fp8) on the Scalar queue; a tiny first
            # slice so the very first matmul can start early ---------------
            we_t = wpool.tile([128, KC, D], FP8, tag="we", name="we")
            for g in range(KG):
                src = weT_d[g].rearrange("p (j q) -> p j q", j=KJ)
                dst = we_t[:, g * KJ:(g + 1) * KJ, :]
                if g == 0:
                    nc.scalar.dma_start(dst[:, :4], src[:, :4])
                    nc.scalar.dma_start(dst[:, 4:], src[:, 4:])
                else:
                    nc.scalar.dma_start(dst, src)

            # small static tensors (Scalar queue)
            R_sb = wpool.tile([128, 2, FC, 128], BF16, tag="Rh", name="R_sb")
            nc.scalar.dma_start(R_sb[:], R_d[:])
            fabD_t = wpool.tile([128, 4, 2, F_PAD], BF16, tag="fabD",
                                name="fabD")
            nc.scalar.dma_start(
                fabD_t[:], fabD_d[:].rearrange("(c p) r f -> p c r f", p=128))
            fabF_t = wpool.tile([128, FC, 2, D], BF16, tag="fabF", name="fabF")
            nc.scalar.dma_start(
                fabF_t[:], fabF_d[:].rearrange("(c p) r d -> p c r d", p=128))
            ones_t = wpool.tile([128, 1], BF16, tag="ones", name="ones")
            nc.scalar.dma_start(ones_t[:], ones_d[:])

            # ---- big-GEMM half pass: 49 DoubleRow chunks x 4 m-tiles ------
            def gemm_half(mat_d, n, tag, first=False, mid_cbs=None):
                accs = [ppool.tile([128, 512], F32, tag="acc",
                                   name=f"acc{tag}{m}") for m in range(4)]
                for g in range(KG):
                    xt = spool.tile([128, KJ, 512], FP8, tag="xs",
                                    name=f"xs{tag}{g}")
                    src = mat_d[n, g].rearrange("p (j q) -> p j q", j=KJ)
                    if first and g == 0:
                        nc.sync.dma_start(xt[:, :4], src[:, :4])
                        nc.sync.dma_start(xt[:, 4:], src[:, 4:])
                    else:
                        nc.sync.dma_start(xt[:], src)
                    for j in range(KJ // 2):
                        kc = g * (KJ // 2) + j
                        for m in range(4):
                            nc.tensor.matmul(
                                accs[m][:],
                                we_t[:, g * KJ + 2 * j:g * KJ + 2 * j + 2,
                                     m * 128:(m + 1) * 128],
                                xt[:, 2 * j:2 * j + 2, :],
                                start=(kc == 0), stop=(kc == NPAIR - 1),
                                perf_mode=DR)
                    if mid_cbs is not None and g in mid_cbs:
                        mid_cbs[g]()
                return accs

            def stage_half(accs, n, tag):
                # stage writes ride the otherwise-idle GPSIMD (SWDGE) queue
                for m in range(4):
                    sb = cpool.tile([128, 512], BF16, tag="cp",
                                    name=f"cp{tag}{m}")
                    nc.vector.tensor_copy(sb[:], accs[m][:])
                    dst = (stage_y[4 * n:4 * n + 4,
                                   m * 128:(m + 1) * 128, :]
                           .rearrange("t d j -> d t j"))
                    nc.gpsimd.dma_start(
                        dst, sb.rearrange("d (t j) -> d t j", t=4))

            # ---- y passes + ReduceScatter --------------------------------
            accs = gemm_half(yT_d, 0, "y0", first=True)
            stage_half(accs, 0, "y0")
            accs = gemm_half(yT_d, 1, "y1")
            stage_half(accs, 1, "y1")
            nc.gpsimd.collective_compute(
                "ReduceScatter", AluOpType.add,
                replica_groups=groups,
                ins=[stage_y[:].opt()],
                outs=[rs_y[:].opt()])

            # ---- x half 0; partial a copied to SBUF to free PSUM ---------
            accs = gemm_half(xT_d, 0, "x0")
            aT0_sb = tpool.tile([128, 4, 512], BF16, name="aT0_sb")
            for m in range(4):
                nc.vector.tensor_copy(aT0_sb[:, m, :], accs[m][:])

            # ---- tail chain: B = rfft(b), P/Q, psi, AllGather ------------
            psi_t = tpool.tile([128, 4, NCORES, 128], BF16, name="psi_t")
            s_sb = tpool.tile([1, B], F32, name="s_sb")

            def tail_chain():
                bT_t = tpool.tile([128, 4, BC], BF16, name="bT_t")
                nc.scalar.dma_start(
                    bT_t[:], rs_y[:].rearrange("(c p) q -> p c q", p=128))
                br_ps = qpool.tile([128, FC, 128], F32, tag="qp", name="br_ps")
                bi_ps = qpool.tile([128, FC, 128], F32, tag="qp", name="bi_ps")
                for ri, ps in ((0, br_ps), (1, bi_ps)):
                    for fc in range(FC):
                        for dc in range(4):
                            nc.tensor.matmul(
                                ps[:, fc, :],
                                fabD_t[:, dc, ri, fc * 128:(fc + 1) * 128],
                                bT_t[:, dc, :],
                                start=(dc == 0), stop=(dc == 3))
                # P = Rr*Br - Ri*Bi ; Q = Rr*Bi + Ri*Br  (convolution)
                t1 = tpool.tile([128, FC, 128], F32, name="t1")
                t2 = tpool.tile([128, FC, 128], F32, name="t2")
                P_sb = tpool.tile([128, FC, 128], BF16, name="P_sb")
                Q_sb = tpool.tile([128, FC, 128], BF16, name="Q_sb")
                nc.vector.tensor_tensor(t1[:], br_ps[:], R_sb[:, 0],
                                        AluOpType.mult)
                nc.vector.tensor_tensor(t2[:], bi_ps[:], R_sb[:, 1],
                                        AluOpType.mult)
                nc.vector.tensor_tensor(P_sb[:], t1[:], t2[:],
                                        AluOpType.subtract)
                nc.vector.tensor_tensor(t1[:], bi_ps[:], R_sb[:, 0],
                                        AluOpType.mult)
                nc.vector.tensor_tensor(t2[:], br_ps[:], R_sb[:, 1],
                                        AluOpType.mult)
                nc.vector.tensor_tensor(Q_sb[:], t1[:], t2[:], AluOpType.add)

                # psi[d,b] = sum_f fabF[f,0,d] P[f,b] + fabF[f,1,d] Q[f,b]
                psi_ps = qpool.tile([128, 4, 128], F32, tag="qp",
                                    name="psi_ps")
                for dc in range(4):
                    step = 0
                    for ri, pq in ((0, P_sb), (1, Q_sb)):
                        for fc in range(FC):
                            nc.tensor.matmul(
                                psi_ps[:, dc, :],
                                fabF_t[:, fc, ri, dc * 128:(dc + 1) * 128],
                                pq[:, fc, :],
                                start=(step == 0), stop=(step == 5))
                            step += 1
                psi_sb = tpool.tile([128, 4, 128], BF16, name="psi_sb")
                nc.vector.tensor_copy(psi_sb[:], psi_ps[:])
                nc.scalar.dma_start(
                    ag_in[:].rearrange("(c p) q -> p c q", p=128), psi_sb[:])
                nc.gpsimd.collective_compute(
                    "AllGather", AluOpType.bypass,
                    replica_groups=groups,
                    ins=[ag_in[:].opt()],
                    outs=[ag_out[:].opt()])
                # gather psi for all 1024 cols; slots 0-3 (x half 0) on
                # Scalar, slots 4-7 (x half 1) on Sync
                for t in range(NCORES):
                    eng = nc.scalar if t < 4 else nc.sync
                    eng.dma_start(
                        psi_t[:, :, t, :],
                        ag_out[t].rearrange("(c p) j -> p c j", p=128))

            def score_half(n, a_srcs):
                s_ps = qpool.tile([1, 512], F32, tag="qp", name=f"s_ps{n}")
                for m in range(4):
                    prod = cpool.tile([128, 512], BF16, tag="cp",
                                      name=f"prod{n}{m}")
                    nc.vector.tensor_tensor(
                        prod[:], a_srcs[m],
                        psi_t[:, m, 4 * n:4 * n + 4, :]
                        .rearrange("p t j -> p (t j)"),
                        AluOpType.mult)
                    nc.tensor.matmul(s_ps[:], ones_t[:], prod[:],
                                     start=(m == 0), stop=(m == 3))
                nc.vector.tensor_copy(s_sb[:, n * 512:(n + 1) * 512], s_ps[:])
                nc.sync.dma_start(out_d[:, n * 512:(n + 1) * 512],
                                  s_sb[:, n * 512:(n + 1) * 512])

            # ---- x half 1 with tail chain + half-0 scores slotted in -----
            accs1 = gemm_half(
                xT_d, 1, "x1",
                mid_cbs={0: tail_chain,
                         6: lambda: score_half(0, [aT0_sb[:, m, :]
                                                   for m in range(4)])})
            score_half(1, [accs1[m][:] for m in range(4)])

    nc.compile()
    return nc


def _get_program():
    if "nc" not in _cached:
        _cached["nc"] = _build_program()
    return _cached["nc"]


def _pack_stream(m8, lo):
    """(B, E)-fp8 matrix -> (2, KG, 128, KJ*512) tile-layout shard."""
    sh = np.zeros((B, E_PAD), dtype=m8.dtype)
    sh[:, :E_SH] = m8[:, lo:lo + E_SH]
    arr = sh.reshape(2, 512, KG, KJ, 128).transpose(0, 2, 4, 3, 1)
    return np.ascontiguousarray(arr).reshape(2, KG, 128, KJ * 512)


def kernel(x, y, r, W_e, W_r):
    nc = _get_program()
    bf = ml_dtypes.bfloat16
    f8 = ml_dtypes.float8_e4m3

    fabD, fabF, fr, fi, fold = _host_consts()

    # host R = rfft(r @ W_r.T) with all constant factors folded (f32 GEMMs)
    rr_full = (r.astype(np.float32) @ W_r.astype(np.float32).T)   # (B, D)
    Rr = rr_full @ (fr * fold).astype(np.float32)                 # (B, NF)
    Ri = rr_full @ (fi * fold).astype(np.float32)
    ones = np.ones((128, 1), dtype=bf)

    x8 = np.clip(x * SX, -240, 240).astype(f8)        # (B, E)
    y8 = np.clip(y * SX, -240, 240).astype(f8)
    w8 = np.clip(W_e * SW, -240, 240).astype(f8)      # (D, E)

    in_maps = []
    for c in range(NCORES):
        lo = c * E_SH
        wsh = np.zeros((D, E_PAD), dtype=f8)
        wsh[:, :E_SH] = w8[:, lo:lo + E_SH]
        # weT[g, p, j*512+q] = W_e.T[(g*KJ+j)*128+p, q]
        warr = wsh.T.reshape(KG, KJ, 128, D).transpose(0, 2, 1, 3)
        # R_d[p, ri, fc, j] = R_ri[fc*128+p, own col j]
        Rh = np.zeros((128, 2, FC, 128), dtype=bf)
        for ri, Rm in ((0, Rr), (1, Ri)):
            own = Rm[c * BC:(c + 1) * BC, :].T        # (NF, 128)
            pad = np.zeros((F_PAD, 128), dtype=np.float32)
            pad[:NF] = own
            Rh[:, ri] = pad.reshape(FC, 128, 128).transpose(1, 0, 2)
        in_maps.append({
            "xT": _pack_stream(x8, lo),
            "yT": _pack_stream(y8, lo),
            "weT": np.ascontiguousarray(warr).reshape(KG, 128, KJ * D),
            "Rh": Rh,
            "fabD": fabD,
            "fabF": fabF,
            "ones": ones,
        })

    res = run_bass_kernel_spmd(nc, in_maps, core_ids=list(range(NCORES)))
    s = np.zeros((B,), dtype=np.float64)
    for c in range(NCORES):
        s += res.results[c]["out"].reshape(B).astype(np.float64)
    out = 1.0 / (1.0 + np.exp(-s))
    return out.reshape(B, 1).astype(np.float32)


# revision 8
# speedup vs baseline: 1.0297x; 1.0297x over previous
"""HolE scorer kernel for 8 Trainium2 NeuronCores (Bass/Tile), fp8 edition.

Computation (reference):
    a = x @ W_e.T; b = y @ W_e.T; rr = r @ W_r.T          # (B, d)
    corr = irfft(rfft(a) * conj(rfft(b))) / d             # circular correlation
    out = sigmoid(sum(rr * corr, axis=1))                 # (B, 1)

Key identity used here: score_i = sum_d a[i,d] * psi[i,d] where
    psi = irfft(rfft(rr) * rfft(b)) / d   (circular convolution dual)
so the score is LINEAR in the per-core partial a's: the x-side GEMM needs
no collective at all - each core emits a partial score vector (1, B) and
the host sums 8 of them (the "unshard" step) and applies the sigmoid.

Strategy:
  - Tensor-parallel over entities: core c holds entity rows
    [c*12500, (c+1)*12500) of x.T/y.T/W_e.T (padded to 12544 = 49*256).
  - Both big GEMMs run in fp8 e4m3 with DoubleRow (double-pumped) matmuls:
    K=256 per instruction at the same 512-column stream rate as bf16.
    Inputs scaled (x*16, W_e*4096) to sit in e4m3's normal range; the
    1/65536^2 descale plus the irfft w_f/d^2 factors are folded into the
    host-side R = rfft(rr) computation.
  - Host pre-packs x/y/W_e shards into the exact SBUF tile layout
    (partition-major, contiguous per partition per group) so every stream
    DMA runs at HBM line rate; R = rfft(r @ W_r.T) is tiny and
    input-independent of the device dataflow, so it is computed on host.
  - y-side: partial b staged bf16 (stage writes on the idle GPSIMD SWDGE
    queue), one ReduceScatter(add) -> core c owns batch cols
    [128c, 128c+128).  B = rfft(b); P/Q complex product; psi via
    irfft-basis matmuls; AllGather psi.  The whole chain is emitted after
    the first k-group of the final x pass, where the RS is guaranteed
    done, so the PE never stalls on it.
  - x-side: partial a stays on-chip; per-core partial scores via
    elementwise mult with psi + ones-vector matmul; (1, B) f32 out per
    core; host sums partials and applies the sigmoid.
"""

import numpy as np
import ml_dtypes

import concourse.bass as bass
import concourse.tile as tile
from concourse import bacc, mybir
from concourse.alu_op_type import AluOpType
from concourse.bass_utils import run_bass_kernel_spmd

# Problem shapes (hardcoded per contract)
B = 1024            # batch
D = 512             # num_dim
E = 100000          # num_entities
R = 1000            # num_relations
NCORES = 8

E_SH = E // NCORES          # 12500 entities per core
KC = 98                     # 128-row k-chunks after padding (98*128 = 12544)
NPAIR = KC // 2             # 49 DoubleRow (K=256) chunks
E_PAD = KC * 128            # 12544
KG = 7                      # stream k-groups
KJ = KC // KG               # 14 chunks per group (7 pairs)
NF = D // 2 + 1             # 257 rfft bins
FC = 3                      # frequency chunks of 128
F_PAD = FC * 128            # 384
BC = B // NCORES            # 128 batch cols owned per core (tail sharding)

SX = 16.0                   # x/y fp8 scale
SW = 4096.0                 # W_e fp8 scale

BF16 = mybir.dt.bfloat16
F32 = mybir.dt.float32
FP8 = mybir.dt.float8e4
DR = mybir.MatmulPerfMode.DoubleRow

_cached = {}


def _host_consts():
    dd = np.arange(D, dtype=np.float64)[:, None]
    ff = np.arange(NF, dtype=np.float64)[None, :]
    ang = 2.0 * np.pi * dd * ff / D
    fr = np.cos(ang)                      # (D, NF)
    fi = -np.sin(ang)
    w = np.full(NF, 2.0); w[0] = 1.0; w[-1] = 1.0
    fold = w / (D * D) / (SX * SW) ** 2

    bf = ml_dtypes.bfloat16
    # d-major rfft basis, f padded to 384: fabD[d, ri, f]
    fabD = np.zeros((D, 2, F_PAD), dtype=bf)
    fabD[:, 0, :NF] = fr.astype(bf)
    fabD[:, 1, :NF] = fi.astype(bf)
    # f-major irfft basis: fabF[f, ri, d]
    fabF = np.zeros((F_PAD, 2, D), dtype=bf)
    fabF[:NF, 0, :] = fr.T.astype(bf)
    fabF[:NF, 1, :] = fi.T.astype(bf)
    return fabD, fabF, fr, fi, fold


def _build_program():
    nc = bacc.Bacc("TRN2", target_bir_lowering=False, debug=False,
                   num_devices=NCORES)

    # stream tensors pre-packed on host into tile layout:
    #   xT[n, g, p, j*512+q] = x.T[core_rows: (g*KJ+j)*128+p, n*512+q] (fp8)
    xT_d = nc.dram_tensor("xT", (2, KG, 128, KJ * 512), FP8,
                          kind="ExternalInput")
    yT_d = nc.dram_tensor("yT", (2, KG, 128, KJ * 512), FP8,
                          kind="ExternalInput")
    weT_d = nc.dram_tensor("weT", (KG, 128, KJ * D), FP8,
                           kind="ExternalInput")
    # host-computed R = rfft(r @ W_r.T) with w_f/d^2/descale folded,
    # f-major, own batch cols: R_d[p, ri, fc, j] = R_ri[fc*128+p, own j]
    R_d = nc.dram_tensor("Rh", (128, 2, FC, 128), BF16, kind="ExternalInput")
    fabD_d = nc.dram_tensor("fabD", (D, 2, F_PAD), BF16, kind="ExternalInput")
    fabF_d = nc.dram_tensor("fabF", (F_PAD, 2, D), BF16, kind="ExternalInput")
    ones_d = nc.dram_tensor("ones", (128, 1), BF16, kind="ExternalInput")
    out_d = nc.dram_tensor("out", (1, B), F32, kind="ExternalOutput")

    stage_y = nc.dram_tensor("stage_y", (NCORES, D, BC), BF16)
    rs_y = nc.dram_tensor("rs_y", (D, BC), BF16)
    ag_in = nc.dram_tensor("ag_in", (D, BC), BF16)
    ag_out = nc.dram_tensor("ag_out", (NCORES, D, BC), BF16,
                            addr_space="Shared")
    groups = [list(range(NCORES))]

    with tile.TileContext(nc) as tc:
        with (
            tc.tile_pool(name="weights", bufs=1) as wpool,
            tc.tile_pool(name="stream", bufs=5) as spool,
            tc.tile_pool(name="copies", bufs=4) as cpool,
            tc.tile_pool(name="tail", bufs=1) as tpool,
            tc.tile_pool(name="psum", bufs=5, space="PSUM") as ppool,
            tc.tile_pool(name="psum_small", bufs=3, space="PSUM") as qpool,
        ):
            # ---- resident W_e.T (fp8) on the Scalar queue; a tiny first
            # slice so the very first matmul can start early ---------------
            we_t = wpool.tile([128, KC, D], FP8, tag="we", name="we")
            for g in range(KG):
                src = weT_d[g].rearrange("p (j q) -> p j q", j=KJ)
                dst = we_t[:, g * KJ:(g + 1) * KJ, :]
                if g == 0:
                    nc.scalar.dma_start(dst[:, :4], src[:, :4])
                    nc.scalar.dma_start(dst[:, 4:], src[:, 4:])
                else:
                    nc.scalar.dma_start(dst, src)

            # small static tensors (Scalar queue)
            R_sb = wpool.tile([128, 2, FC, 128], BF16, tag="Rh", name="R_sb")
            nc.scalar.dma_start(R_sb[:], R_d[:])
            fabD_t = wpool.tile([128, 4, 2, F_PAD], BF16, tag="fabD",
                                name="fabD")
            nc.scalar.dma_start(
                fabD_t[:], fabD_d[:].rearrange("(c p) r f -> p c r f", p=128))
            fabF_t = wpool.tile([128, FC, 2, D], BF16, tag="fabF", name="fabF")
            nc.scalar.dma_start(
                fabF_t[:], fabF_d[:].rearrange("(c p) r d -> p c r d", p=128))
            ones_t = wpool.tile([128, 1], BF16, tag="ones", name="ones")
            nc.scalar.dma_start(ones_t[:], ones_d[:])

            # ---- big-GEMM half pass: 49 DoubleRow chunks x 4 m-tiles ------
            def gemm_half(mat_d, n, tag, first=False, mid_cbs=None):
                accs = [ppool.tile([128, 512], F32, tag="acc",
                                   name=f"acc{tag}{m}") for m in range(4)]
                for g in range(KG):
                    xt = spool.tile([128, KJ, 512], FP8, tag="xs",
                                    name=f"xs{tag}{g}")
                    src = mat_d[n, g].rearrange("p (j q) -> p j q", j=KJ)
                    if first and g == 0:
                        nc.sync.dma_start(xt[:, :4], src[:, :4])
                        nc.sync.dma_start(xt[:, 4:], src[:, 4:])
                    else:
                        nc.sync.dma_start(xt[:], src)
                    for j in range(KJ // 2):
                        kc = g * (KJ // 2) + j
                        for m in range(4):
                            nc.tensor.matmul(
                                accs[m][:],
                                we_t[:, g * KJ + 2 * j:g * KJ + 2 * j + 2,
                                     m * 128:(m + 1) * 128],
                                xt[:, 2 * j:2 * j + 2, :],
                                start=(kc == 0), stop=(kc == NPAIR - 1),
                                perf_mode=DR)
                    if mid_cbs is not None and g in mid_cbs:
                        mid_cbs[g]()
                return accs

            def stage_half(accs, n, tag):
                # stage writes ride the otherwise-idle GPSIMD (SWDGE) queue
                for m in range(4):
                    sb = cpool.tile([128, 512], BF16, tag="cp",
                                    name=f"cp{tag}{m}")
                    nc.vector.tensor_copy(sb[:], accs[m][:])
                    dst = (stage_y[4 * n:4 * n + 4,
                                   m * 128:(m + 1) * 128, :]
                           .rearrange("t d j -> d t j"))
                    nc.gpsimd.dma_start(
                        dst, sb.rearrange("d (t j) -> d t j", t=4))

            # ---- y passes + ReduceScatter --------------------------------
            accs = gemm_half(yT_d, 0, "y0", first=True)
            stage_half(accs, 0, "y0")
            accs = gemm_half(yT_d, 1, "y1")
            stage_half(accs, 1, "y1")
            nc.gpsimd.collective_compute(
                "ReduceScatter", AluOpType.add,
                replica_groups=groups,
                ins=[stage_y[:].opt()],
                outs=[rs_y[:].opt()])

            # ---- x half 0; partial a copied to SBUF to free PSUM ---------
            accs = gemm_half(xT_d, 0, "x0")
            aT0_sb = tpool.tile([128, 4, 512], BF16, name="aT0_sb")
            for m in range(4):
                nc.vector.tensor_copy(aT0_sb[:, m, :], accs[m][:])

            # ---- tail chain: B = rfft(b), P/Q, psi, AllGather ------------
            psi_t = tpool.tile([128, 4, NCORES, 128], BF16, name="psi_t")
            s_sb = tpool.tile([1, B], F32, name="s_sb")

            def tail_chain():
                bT_t = tpool.tile([128, 4, BC], BF16, name="bT_t")
                nc.scalar.dma_start(
                    bT_t[:], rs_y[:].rearrange("(c p) q -> p c q", p=128))
                br_ps = qpool.tile([128, FC, 128], F32, tag="qp", name="br_ps")
                bi_ps = qpool.tile([128, FC, 128], F32, tag="qp", name="bi_ps")
                for ri, ps in ((0, br_ps), (1, bi_ps)):
                    for fc in range(FC):
                        for dc in range(4):
                            nc.tensor.matmul(
                                ps[:, fc, :],
                                fabD_t[:, dc, ri, fc * 128:(fc + 1) * 128],
                                bT_t[:, dc, :],
                                start=(dc == 0), stop=(dc == 3))
                # P = Rr*Br - Ri*Bi ; Q = Rr*Bi + Ri*Br  (convolution)
                t1 = tpool.tile([128, FC, 128], F32, name="t1")
                t2 = tpool.tile([128, FC, 128], F32, name="t2")
                P_sb = tpool.tile([128, FC, 128], BF16, name="P_sb")
                Q_sb = tpool.tile([128, FC, 128], BF16, name="Q_sb")
                nc.vector.tensor_tensor(t1[:], br_ps[:], R_sb[:, 0],
                                        AluOpType.mult)
                nc.vector.tensor_tensor(t2[:], bi_ps[:], R_sb[:, 1],
                                        AluOpType.mult)
                nc.vector.tensor_tensor(P_sb[:], t1[:], t2[:],
                                        AluOpType.subtract)
                nc.vector.tensor_tensor(t1[:], bi_ps[:], R_sb[:, 0],
                                        AluOpType.mult)
                nc.vector.tensor_tensor(t2[:], br_ps[:], R_sb[:, 1],
                                        AluOpType.mult)
                nc.vector.tensor_tensor(Q_sb[:], t1[:], t2[:], AluOpType.add)

                # psi[d,b] = sum_f fabF[f,0,d] P[f,b] + fabF[f,1,d] Q[f,b]
                psi_ps = qpool.tile([128, 4, 128], F32, tag="qp",
                                    name="psi_ps")
                for dc in range(4):
                    step = 0
                    for ri, pq in ((0, P_sb), (1, Q_sb)):
                        for fc in range(FC):
                            nc.tensor.matmul(
                                psi_ps[:, dc, :],
                                fabF_t[:, fc, ri, dc * 128:(dc + 1) * 128],
                                pq[:, fc, :],
                                start=(step == 0), stop=(step == 5))
                            step += 1
                psi_sb = tpool.tile([128, 4, 128], BF16, name="psi_sb")
                nc.vector.tensor_copy(psi_sb[:], psi_ps[:])
                nc.scalar.dma_start(
                    ag_in[:].rearrange("(c p) q -> p c q", p=128), psi_sb[:])
                nc.gpsimd.collective_compute(
                    "AllGather", AluOpType.bypass,
                    replica_groups=groups,
                    ins=[ag_in[:].opt()],
                    outs=[ag_out[:].opt()])
                # gather psi for all 1024 cols; slots 0-3 (x half 0) on
                # Scalar, slots 4-7 (x half 1) on Sync
                for t in range(NCORES):
                    eng = nc.scalar if t < 4 else nc.sync
                    eng.dma_start(
                        psi_t[:, :, t, :],
                        ag_out[t].rearrange("(c p) j -> p c j", p=128))

            def score_half(n, a_srcs):
                s_ps = qpool.tile([1, 512], F32, tag="qp", name=f"s_ps{n}")
                for m in range(4):
                    prod = cpool.tile([128, 512], BF16, tag="cp",
                                      name=f"prod{n}{m}")
                    nc.vector.tensor_tensor(
                        prod[:], a_srcs[m],
                        psi_t[:, m, 4 * n:4 * n + 4, :]
                        .rearrange("p t j -> p (t j)"),
                        AluOpType.mult)
                    nc.tensor.matmul(s_ps[:], ones_t[:], prod[:],
                                     start=(m == 0), stop=(m == 3))
                nc.vector.tensor_copy(s_sb[:, n * 512:(n + 1) * 512], s_ps[:])
                nc.sync.dma_start(out_d[:, n * 512:(n + 1) * 512],
                                  s_sb[:, n * 512:(n + 1) * 512])

            # ---- x half 1 with tail chain + half-0 scores slotted in -----
            accs1 = gemm_half(xT_d, 1, "x1", mid_cbs={2: tail_chain})
            score_half(0, [aT0_sb[:, m, :] for m in range(4)])
            score_half(1, [accs1[m][:] for m in range(4)])

    nc.compile()
    return nc


def _get_program():
    if "nc" not in _cached:
        _cached["nc"] = _build_program()
    return _cached["nc"]


def _pack_stream(m8, lo):
    """(B, E)-fp8 matrix -> (2, KG, 128, KJ*512) tile-layout shard."""
    sh = np.zeros((B, E_PAD), dtype=m8.dtype)
    sh[:, :E_SH] = m8[:, lo:lo + E_SH]
    arr = sh.reshape(2, 512, KG, KJ, 128).transpose(0, 2, 4, 3, 1)
    return np.ascontiguousarray(arr).reshape(2, KG, 128, KJ * 512)


def kernel(x, y, r, W_e, W_r):
    nc = _get_program()
    bf = ml_dtypes.bfloat16
    f8 = ml_dtypes.float8_e4m3

    fabD, fabF, fr, fi, fold = _host_consts()

    # host R = rfft(r @ W_r.T) with all constant factors folded (f32 GEMMs)
    rr_full = (r.astype(np.float32) @ W_r.astype(np.float32).T)   # (B, D)
    Rr = rr_full @ (fr * fold).astype(np.float32)                 # (B, NF)
    Ri = rr_full @ (fi * fold).astype(np.float32)
    ones = np.ones((128, 1), dtype=bf)

    x8 = np.clip(x * SX, -240, 240).astype(f8)        # (B, E)
    y8 = np.clip(y * SX, -240, 240).astype(f8)
    w8 = np.clip(W_e * SW, -240, 240).astype(f8)      # (D, E)

    in_maps = []
    for c in range(NCORES):
        lo = c * E_SH
        wsh = np.zeros((D, E_PAD), dtype=f8)
        wsh[:, :E_SH] = w8[:, lo:lo + E_SH]
        # weT[g, p, j*512+q] = W_e.T[(g*KJ+j)*128+p, q]
        warr = wsh.T.reshape(KG, KJ, 128, D).transpose(0, 2, 1, 3)
        # R_d[p, ri, fc, j] = R_ri[fc*128+p, own col j]
        Rh = np.zeros((128, 2, FC, 128), dtype=bf)
        for ri, Rm in ((0, Rr), (1, Ri)):
            own = Rm[c * BC:(c + 1) * BC, :].T        # (NF, 128)
            pad = np.zeros((F_PAD, 128), dtype=np.float32)
            pad[:NF] = own
            Rh[:, ri] = pad.reshape(FC, 128, 128).transpose(1, 0, 2)
        in_maps.append({
            "xT": _pack_stream(x8, lo),
            "yT": _pack_stream(y8, lo),
            "weT": np.ascontiguousarray(warr).reshape(KG, 128, KJ * D),
            "Rh": Rh,
            "fabD": fabD,
            "fabF": fabF,
            "ones": ones,
        })

    res = run_bass_kernel_spmd(nc, in_maps, core_ids=list(range(NCORES)))
    s = np.zeros((B,), dtype=np.float64)
    for c in range(NCORES):
        s += res.results[c]["out"].reshape(B).astype(np.float64)
    out = 1.0 / (1.0 + np.exp(-s))
    return out.reshape(B, 1).astype(np.float32)


# revision 9
# speedup vs baseline: 1.0694x; 1.0386x over previous
"""HolE scorer kernel for 8 Trainium2 NeuronCores (Bass/Tile), fp8 edition.

Computation (reference):
    a = x @ W_e.T; b = y @ W_e.T; rr = r @ W_r.T          # (B, d)
    corr = irfft(rfft(a) * conj(rfft(b))) / d             # circular correlation
    out = sigmoid(sum(rr * corr, axis=1))                 # (B, 1)

Key identity used here: score_i = sum_d a[i,d] * psi[i,d] where
    psi = irfft(rfft(rr) * rfft(b)) / d   (circular convolution dual)
so the score is LINEAR in the per-core partial a's: the x-side GEMM needs
no collective at all - each core emits a partial score vector (1, B) and
the host sums 8 of them (the "unshard" step) and applies the sigmoid.

Strategy:
  - Tensor-parallel over entities: core c holds entity rows
    [c*12500, (c+1)*12500) of x.T/y.T/W_e.T (padded to 12544 = 49*256).
  - Both big GEMMs run in fp8 e4m3 with DoubleRow (double-pumped) matmuls:
    K=256 per instruction at the same 512-column stream rate as bf16.
    Inputs scaled (x*16, W_e*4096) to sit in e4m3's normal range; the
    1/65536^2 descale plus the irfft w_f/d^2 factors are folded into the
    host-side R = rfft(rr) computation.
  - Host pre-packs x/y/W_e shards into the exact SBUF tile layout
    (partition-major, contiguous per partition per group) so every stream
    DMA runs at HBM line rate; R = rfft(r @ W_r.T) is tiny and
    input-independent of the device dataflow, so it is computed on host.
  - y-side: partial b staged bf16 (stage writes on the idle GPSIMD SWDGE
    queue), one ReduceScatter(add) -> core c owns batch cols
    [128c, 128c+128).  B = rfft(b); P/Q complex product; psi via
    irfft-basis matmuls; AllGather psi.  The whole chain is emitted after
    the first k-group of the final x pass, where the RS is guaranteed
    done, so the PE never stalls on it.
  - x-side: partial a stays on-chip; per-core partial scores via
    elementwise mult with psi + ones-vector matmul; (1, B) f32 out per
    core; host sums partials and applies the sigmoid.
"""

import numpy as np
import ml_dtypes

import concourse.bass as bass
import concourse.tile as tile
from concourse import bacc, mybir
from concourse.alu_op_type import AluOpType
from concourse.bass_utils import run_bass_kernel_spmd

# Problem shapes (hardcoded per contract)
B = 1024            # batch
D = 512             # num_dim
E = 100000          # num_entities
R = 1000            # num_relations
NCORES = 8

E_SH = E // NCORES          # 12500 entities per core
KC = 98                     # 128-row k-chunks after padding (98*128 = 12544)
NPAIR = KC // 2             # 49 DoubleRow (K=256) chunks
E_PAD = KC * 128            # 12544
KG = 7                      # stream k-groups
KJ = KC // KG               # 14 chunks per group (7 pairs)
NF = D // 2 + 1             # 257 rfft bins
FC = 3                      # frequency chunks of 128
F_PAD = FC * 128            # 384
BC = B // NCORES            # 128 batch cols owned per core (tail sharding)

SX = 16.0                   # x/y fp8 scale
SW = 4096.0                 # W_e fp8 scale

BF16 = mybir.dt.bfloat16
F32 = mybir.dt.float32
FP8 = mybir.dt.float8e4
DR = mybir.MatmulPerfMode.DoubleRow

_cached = {}


def _host_consts():
    dd = np.arange(D, dtype=np.float64)[:, None]
    ff = np.arange(NF, dtype=np.float64)[None, :]
    ang = 2.0 * np.pi * dd * ff / D
    fr = np.cos(ang)                      # (D, NF)
    fi = -np.sin(ang)
    w = np.full(NF, 2.0); w[0] = 1.0; w[-1] = 1.0
    fold = w / (D * D) / (SX * SW) ** 2

    bf = ml_dtypes.bfloat16
    # d-major rfft basis, f padded to 384: fabD[d, ri, f]
    fabD = np.zeros((D, 2, F_PAD), dtype=bf)
    fabD[:, 0, :NF] = fr.astype(bf)
    fabD[:, 1, :NF] = fi.astype(bf)
    # f-major irfft basis: fabF[f, ri, d]
    fabF = np.zeros((F_PAD, 2, D), dtype=bf)
    fabF[:NF, 0, :] = fr.T.astype(bf)
    fabF[:NF, 1, :] = fi.T.astype(bf)
    return fabD, fabF, fr, fi, fold


def _build_program():
    nc = bacc.Bacc("TRN2", target_bir_lowering=False, debug=False,
                   num_devices=NCORES)

    # stream tensors pre-packed on host into tile layout:
    #   xT[n, g, p, j*512+q] = x.T[core_rows: (g*KJ+j)*128+p, n*512+q] (fp8)
    xT_d = nc.dram_tensor("xT", (2, KG, 128, KJ * 512), FP8,
                          kind="ExternalInput")
    yT_d = nc.dram_tensor("yT", (2, KG, 128, KJ * 512), FP8,
                          kind="ExternalInput")
    weT_d = nc.dram_tensor("weT", (KG, 128, KJ * D), FP8,
                           kind="ExternalInput")
    # host-computed R = rfft(r @ W_r.T) with w_f/d^2/descale folded,
    # f-major, own batch cols: R_d[p, ri, fc, j] = R_ri[fc*128+p, own j]
    R_d = nc.dram_tensor("Rh", (128, 2, FC, 128), BF16, kind="ExternalInput")
    fabD_d = nc.dram_tensor("fabD", (D, 2, F_PAD), BF16, kind="ExternalInput")
    fabF_d = nc.dram_tensor("fabF", (F_PAD, 2, D), BF16, kind="ExternalInput")
    ones_d = nc.dram_tensor("ones", (128, 1), BF16, kind="ExternalInput")
    out_d = nc.dram_tensor("out", (1, B), F32, kind="ExternalOutput")

    stage_y = nc.dram_tensor("stage_y", (NCORES, D, BC), BF16)
    rs_y = nc.dram_tensor("rs_y", (D, BC), BF16)
    ag_in = nc.dram_tensor("ag_in", (D, BC), BF16)
    ag_out = nc.dram_tensor("ag_out", (NCORES, D, BC), BF16,
                            addr_space="Shared")
    groups = [list(range(NCORES))]

    with tile.TileContext(nc) as tc:
        with (
            tc.tile_pool(name="weights", bufs=1) as wpool,
            tc.tile_pool(name="stream", bufs=5) as spool,
            tc.tile_pool(name="copies", bufs=4) as cpool,
            tc.tile_pool(name="tail", bufs=1) as tpool,
            tc.tile_pool(name="psum", bufs=5, space="PSUM") as ppool,
            tc.tile_pool(name="psum_small", bufs=3, space="PSUM") as qpool,
        ):
            # ---- resident W_e.T (fp8) on the Scalar queue; a tiny first
            # slice so the very first matmul can start early ---------------
            we_t = wpool.tile([128, KC, D], FP8, tag="we", name="we")
            for g in range(KG):
                src = weT_d[g].rearrange("p (j q) -> p j q", j=KJ)
                dst = we_t[:, g * KJ:(g + 1) * KJ, :]
                if g == 0:
                    nc.scalar.dma_start(dst[:, :4], src[:, :4])
                    nc.scalar.dma_start(dst[:, 4:], src[:, 4:])
                else:
                    nc.scalar.dma_start(dst, src)

            # small static tensors (Scalar queue)
            R_sb = wpool.tile([128, 2, FC, 128], BF16, tag="Rh", name="R_sb")
            nc.scalar.dma_start(R_sb[:], R_d[:])
            fabD_t = wpool.tile([128, 4, 2, F_PAD], BF16, tag="fabD",
                                name="fabD")
            nc.scalar.dma_start(
                fabD_t[:], fabD_d[:].rearrange("(c p) r f -> p c r f", p=128))
            fabF_t = wpool.tile([128, FC, 2, D], BF16, tag="fabF", name="fabF")
            nc.scalar.dma_start(
                fabF_t[:], fabF_d[:].rearrange("(c p) r d -> p c r d", p=128))
            ones_t = wpool.tile([128, 1], BF16, tag="ones", name="ones")
            nc.scalar.dma_start(ones_t[:], ones_d[:])

            # ---- big-GEMM half pass: 49 DoubleRow chunks x 4 m-tiles ------
            def gemm_half(mat_d, n, tag, first=False, mid_cbs=None):
                accs = [ppool.tile([128, 512], F32, tag="acc",
                                   name=f"acc{tag}{m}") for m in range(4)]
                for g in range(KG):
                    xt = spool.tile([128, KJ, 512], FP8, tag="xs",
                                    name=f"xs{tag}{g}")
                    src = mat_d[n, g].rearrange("p (j q) -> p j q", j=KJ)
                    if first and g == 0:
                        nc.sync.dma_start(xt[:, :4], src[:, :4])
                        nc.sync.dma_start(xt[:, 4:], src[:, 4:])
                    else:
                        nc.sync.dma_start(xt[:], src)
                    for j in range(KJ // 2):
                        kc = g * (KJ // 2) + j
                        for m in range(4):
                            nc.tensor.matmul(
                                accs[m][:],
                                we_t[:, g * KJ + 2 * j:g * KJ + 2 * j + 2,
                                     m * 128:(m + 1) * 128],
                                xt[:, 2 * j:2 * j + 2, :],
                                start=(kc == 0), stop=(kc == NPAIR - 1),
                                perf_mode=DR)
                    if mid_cbs is not None and g in mid_cbs:
                        mid_cbs[g]()
                return accs

            def stage_half(accs, n, tag):
                # stage writes ride the otherwise-idle GPSIMD (SWDGE) queue
                for m in range(4):
                    sb = cpool.tile([128, 512], BF16, tag="cp",
                                    name=f"cp{tag}{m}")
                    nc.vector.tensor_copy(sb[:], accs[m][:])
                    dst = (stage_y[4 * n:4 * n + 4,
                                   m * 128:(m + 1) * 128, :]
                           .rearrange("t d j -> d t j"))
                    nc.gpsimd.dma_start(
                        dst, sb.rearrange("d (t j) -> d t j", t=4))

            # ---- y passes + ReduceScatter --------------------------------
            accs = gemm_half(yT_d, 0, "y0", first=True)
            stage_half(accs, 0, "y0")
            accs = gemm_half(yT_d, 1, "y1")
            stage_half(accs, 1, "y1")
            nc.gpsimd.collective_compute(
                "ReduceScatter", AluOpType.add,
                replica_groups=groups,
                ins=[stage_y[:].opt()],
                outs=[rs_y[:].opt()])

            # ---- x half 0; partial a copied to SBUF to free PSUM ---------
            accs = gemm_half(xT_d, 0, "x0")
            aT0_sb = tpool.tile([128, 4, 512], BF16, name="aT0_sb")
            for m in range(4):
                nc.vector.tensor_copy(aT0_sb[:, m, :], accs[m][:])

            # ---- tail chain: B = rfft(b), P/Q, psi, AllGather ------------
            psi_t = tpool.tile([128, 4, NCORES, 128], BF16, name="psi_t")
            s_sb = tpool.tile([1, B], F32, name="s_sb")

            def tail_chain():
                bT_t = tpool.tile([128, 4, BC], BF16, name="bT_t")
                nc.scalar.dma_start(
                    bT_t[:], rs_y[:].rearrange("(c p) q -> p c q", p=128))
                br_ps = qpool.tile([128, FC, 128], F32, tag="qp", name="br_ps")
                bi_ps = qpool.tile([128, FC, 128], F32, tag="qp", name="bi_ps")
                for ri, ps in ((0, br_ps), (1, bi_ps)):
                    for fc in range(FC):
                        for dc in range(4):
                            nc.tensor.matmul(
                                ps[:, fc, :],
                                fabD_t[:, dc, ri, fc * 128:(fc + 1) * 128],
                                bT_t[:, dc, :],
                                start=(dc == 0), stop=(dc == 3))
                # P = Rr*Br - Ri*Bi ; Q = Rr*Bi + Ri*Br  (convolution)
                t1 = tpool.tile([128, FC, 128], F32, name="t1")
                t2 = tpool.tile([128, FC, 128], F32, name="t2")
                P_sb = tpool.tile([128, FC, 128], BF16, name="P_sb")
                Q_sb = tpool.tile([128, FC, 128], BF16, name="Q_sb")
                nc.vector.tensor_tensor(t1[:], br_ps[:], R_sb[:, 0],
                                        AluOpType.mult)
                nc.vector.tensor_tensor(t2[:], bi_ps[:], R_sb[:, 1],
                                        AluOpType.mult)
                nc.vector.tensor_tensor(P_sb[:], t1[:], t2[:],
                                        AluOpType.subtract)
                nc.vector.tensor_tensor(t1[:], bi_ps[:], R_sb[:, 0],
                                        AluOpType.mult)
                nc.vector.tensor_tensor(t2[:], br_ps[:], R_sb[:, 1],
                                        AluOpType.mult)
                nc.vector.tensor_tensor(Q_sb[:], t1[:], t2[:], AluOpType.add)

                # psi[d,b] = sum_f fabF[f,0,d] P[f,b] + fabF[f,1,d] Q[f,b]
                psi_ps = qpool.tile([128, 4, 128], F32, tag="qp",
                                    name="psi_ps")
                for dc in range(4):
                    step = 0
                    for ri, pq in ((0, P_sb), (1, Q_sb)):
                        for fc in range(FC):
                            nc.tensor.matmul(
                                psi_ps[:, dc, :],
                                fabF_t[:, fc, ri, dc * 128:(dc + 1) * 128],
                                pq[:, fc, :],
                                start=(step == 0), stop=(step == 5))
                            step += 1
                psi_sb = tpool.tile([128, 4, 128], BF16, name="psi_sb")
                nc.vector.tensor_copy(psi_sb[:], psi_ps[:])
                nc.gpsimd.dma_start(
                    ag_in[:].rearrange("(c p) q -> p c q", p=128), psi_sb[:])
                nc.gpsimd.collective_compute(
                    "AllGather", AluOpType.bypass,
                    replica_groups=groups,
                    ins=[ag_in[:].opt()],
                    outs=[ag_out[:].opt()])
                # gather psi for all 1024 cols; slots 0-3 (x half 0) on
                # Scalar, slots 4-7 (x half 1) on Sync
                engs = [nc.scalar, nc.scalar, nc.scalar, nc.gpsimd,
                        nc.sync, nc.sync, nc.sync, nc.gpsimd]
                for t in range(NCORES):
                    engs[t].dma_start(
                        psi_t[:, :, t, :],
                        ag_out[t].rearrange("(c p) j -> p c j", p=128))

            def score_half(n, a_srcs):
                s_ps = qpool.tile([1, 512], F32, tag="qp", name=f"s_ps{n}")
                for m in range(4):
                    prod = cpool.tile([128, 512], BF16, tag="cp",
                                      name=f"prod{n}{m}")
                    nc.vector.tensor_tensor(
                        prod[:], a_srcs[m],
                        psi_t[:, m, 4 * n:4 * n + 4, :]
                        .rearrange("p t j -> p (t j)"),
                        AluOpType.mult)
                    nc.tensor.matmul(s_ps[:], ones_t[:], prod[:],
                                     start=(m == 0), stop=(m == 3))
                nc.vector.tensor_copy(s_sb[:, n * 512:(n + 1) * 512], s_ps[:])
                nc.sync.dma_start(out_d[:, n * 512:(n + 1) * 512],
                                  s_sb[:, n * 512:(n + 1) * 512])

            # ---- x half 1 with tail chain + half-0 scores slotted in -----
            accs1 = gemm_half(xT_d, 1, "x1", mid_cbs={0: tail_chain})
            score_half(0, [aT0_sb[:, m, :] for m in range(4)])
            score_half(1, [accs1[m][:] for m in range(4)])

    nc.compile()
    return nc


def _get_program():
    if "nc" not in _cached:
        _cached["nc"] = _build_program()
    return _cached["nc"]


def _pack_stream(m8, lo):
    """(B, E)-fp8 matrix -> (2, KG, 128, KJ*512) tile-layout shard."""
    sh = np.zeros((B, E_PAD), dtype=m8.dtype)
    sh[:, :E_SH] = m8[:, lo:lo + E_SH]
    arr = sh.reshape(2, 512, KG, KJ, 128).transpose(0, 2, 4, 3, 1)
    return np.ascontiguousarray(arr).reshape(2, KG, 128, KJ * 512)


def kernel(x, y, r, W_e, W_r):
    nc = _get_program()
    bf = ml_dtypes.bfloat16
    f8 = ml_dtypes.float8_e4m3

    fabD, fabF, fr, fi, fold = _host_consts()

    # host R = rfft(r @ W_r.T) with all constant factors folded (f32 GEMMs)
    rr_full = (r.astype(np.float32) @ W_r.astype(np.float32).T)   # (B, D)
    Rr = rr_full @ (fr * fold).astype(np.float32)                 # (B, NF)
    Ri = rr_full @ (fi * fold).astype(np.float32)
    ones = np.ones((128, 1), dtype=bf)

    x8 = np.clip(x * SX, -240, 240).astype(f8)        # (B, E)
    y8 = np.clip(y * SX, -240, 240).astype(f8)
    w8 = np.clip(W_e * SW, -240, 240).astype(f8)      # (D, E)

    in_maps = []
    for c in range(NCORES):
        lo = c * E_SH
        wsh = np.zeros((D, E_PAD), dtype=f8)
        wsh[:, :E_SH] = w8[:, lo:lo + E_SH]
        # weT[g, p, j*512+q] = W_e.T[(g*KJ+j)*128+p, q]
        warr = wsh.T.reshape(KG, KJ, 128, D).transpose(0, 2, 1, 3)
        # R_d[p, ri, fc, j] = R_ri[fc*128+p, own col j]
        Rh = np.zeros((128, 2, FC, 128), dtype=bf)
        for ri, Rm in ((0, Rr), (1, Ri)):
            own = Rm[c * BC:(c + 1) * BC, :].T        # (NF, 128)
            pad = np.zeros((F_PAD, 128), dtype=np.float32)
            pad[:NF] = own
            Rh[:, ri] = pad.reshape(FC, 128, 128).transpose(1, 0, 2)
        in_maps.append({
            "xT": _pack_stream(x8, lo),
            "yT": _pack_stream(y8, lo),
            "weT": np.ascontiguousarray(warr).reshape(KG, 128, KJ * D),
            "Rh": Rh,
            "fabD": fabD,
            "fabF": fabF,
            "ones": ones,
        })

    res = run_bass_kernel_spmd(nc, in_maps, core_ids=list(range(NCORES)))
    s = np.zeros((B,), dtype=np.float64)
    for c in range(NCORES):
        s += res.results[c]["out"].reshape(B).astype(np.float64)
    out = 1.0 / (1.0 + np.exp(-s))
    return out.reshape(B, 1).astype(np.float32)
